# revision 22
# baseline (speedup 1.0000x reference)
"""Trainium2 Bass kernel for 16-head MHA (B=2, T=2048, E=1024), SPMD on 8 cores.

Sharding: data-parallel over batch (2) x tensor-parallel over head groups
(4 groups of 4 heads). Each core computes q/k/v projections for its 4 heads,
exact-max shifted softmax attention, and a partial out-projection over its
256 embedding columns; the host sums the 4 partials per batch.

All matmuls run as float32r (fp32 data truncated to fp22 in the PE) which
streams at 1 cycle/row for moving dims >= 256 -- 4x the fp32 rate. Measured
end-to-end rel err vs the fp32 reference: ~2.6e-3 (tolerance 2e-2); bf16
fails (3.7e-2) because the softmax here is near-one-hot with huge scores.

Per-head softmax shift: pass 1 computes scores [i-part, j-free] (K=64
matmuls) and reduces the row max with fused tensor_tensor_reduce ops that
read TWO PSUM banks per pass (op0=max of the banks, scale=-1, op1=min with
chained init => negated row max in one go). A PE transpose lays -M along
the free dim of q_aug row 64, so the main K=65 augmented QK matmul lands
scores in PSUM already shifted. exp runs on ACT over [128,1024] (2 banks)
straight out of PSUM; the softmax denominator comes from a ones column
appended to V; normalization uses reciprocal_approx_fast + a rank-1 PE
broadcast matmul.
"""

import sys

sys.path.insert(0, "/opt/trn_rl_repo")

import numpy as np

import concourse.bass as bass
import concourse.mybir as mybir
import concourse.tile as tile_mod
from concourse.masks import make_identity

F32 = mybir.dt.float32
F16 = mybir.dt.float16

B, T, E = 2, 2048, 1024
H_TOTAL, D = 16, 64
N_CORES = 8
GROUPS = 4
HPG = H_TOTAL // GROUPS     # 4 heads per core
DV = HPG * D                # 256 v width / out-proj contraction per core
FQK = 2 * DV                # 512 q+k feature rows per core
SCALE = float(np.sqrt(D))   # reference MULTIPLIES scores by sqrt(d)

NE = E // 128               # 8 e-chunks
NT_TILE = T // 128          # 16 token tiles
NT_CHUNK = T // 512         # 4 query chunks
NEG_INIT = 3.0e38           # init for the min-reduce of negated maxes


def _f32r(ap):
    return ap


# ---------------------------------------------------------------------------
# Workaround: this walrus build only accepts ONE sem wait per instruction.
def _split_multi_waits(nc):
    for f in nc.m.functions:
        for bb in f.blocks:
            out = []
            for inst in bb.instructions:
                si = getattr(inst, "sync_info", None)
                if si is not None and si.on_wait and len(si.on_wait) > 1:
                    extras = list(si.on_wait[:-1])
                    si.on_wait = list(si.on_wait[-1:])
                    for w in extras:
                        nop = mybir.InstNoOp(
                            name=f"I-{nc.next_id()}", ins=[], outs=[]
                        )
                        nop.engine = inst.engine
                        nop.sync_info = mybir.SyncInfo(on_wait=[w], on_update=[])
                        out.append(nop)
                out.append(inst)
            bb.instructions[:] = out


# ---------------------------------------------------------------------------
def _emit_body(nc, tc, dram, ctx_pools, dbg=None):
    xT_d, wqkT_d, wvT_d, woutT_d, y_d = dram
    persist = ctx_pools["persist"]

    # --- persistent SBUF (fp16 operands) -----------------------------------
    q_aug = [persist.tile([D + 1, T], F16, tag=f"qaug{h}", name=f"qaug{h}")
             for h in range(HPG)]
    k_aug = [persist.tile([D + 1, T], F16, tag=f"kaug{h}", name=f"kaug{h}")
             for h in range(HPG)]
    # all heads' V in one tile: per (h, jt) 65 cols = [v_h(64) | 1]
    vaug = persist.tile([128, HPG * NT_TILE * (D + 1)], F16, tag="vaug",
                        name="vaug")
    # raw projection tiles [q0;q1] [k0;k1] [q2;q3] [k2;k3] (kept for the
    # packed max pass which reads both 64-partition halves)
    qk_sb = [persist.tile([128, T], F16, tag=f"qk{i}", name=f"qk{i}")
             for i in range(FQK // 128)]
    ones_t = persist.tile([1, D], F16, tag="ones_t", name="ones_t")
    nc.vector.memset(ones_t, 1.0)
    identity = persist.tile([128, 128], F32, tag="identity", name="identity")
    make_identity(nc, identity)
    va_r = vaug.rearrange("p (s c) -> p s c", c=D + 1)
    nc.vector.memset(va_r[:, :, D:D + 1], 1.0)
    for h in range(HPG):
        nc.vector.memset(k_aug[h][D:D + 1, :], 1.0)

    def vaug_sl(h, jt):
        base = (h * NT_TILE + jt) * (D + 1)
        return vaug[:, base:base + D + 1]

    # --- Phase 1: projections ----------------------------------------------
    with (
        tc.tile_pool(name="ph1", bufs=1) as ph1,
        tc.tile_pool(name="pj", bufs=4, space="PSUM") as pj,
        tc.tile_pool(name="pv", bufs=2, space="PSUM") as pvp,
    ):
        xt_sb = [ph1.tile([128, T], F16, tag=f"xt{i}", name=f"xt{i}")
                 for i in range(NE)]
        wqk_sb = [ph1.tile([128, FQK], F16, tag=f"wqk{i}", name=f"wqk{i}")
                  for i in range(NE)]
        wv_sb = [ph1.tile([128, DV], F16, tag=f"wv{i}", name=f"wv{i}")
                 for i in range(NE)]
        for i in range(NE):
            nc.sync.dma_start(out=xt_sb[i], in_=xT_d[i * 128:(i + 1) * 128, :])
            nc.sync.dma_start(out=wqk_sb[i], in_=wqkT_d[i * 128:(i + 1) * 128, :])
            nc.sync.dma_start(out=wv_sb[i], in_=wvT_d[i * 128:(i + 1) * 128, :])

        # qk^T feature-major; PSUM->SBUF copies on ACT (idle in phase 1)
        for ff in range(FQK // 128):
            ps = [pj.tile([128, 512], F32, tag="pj", name="pj")
                  for _ in range(NT_CHUNK)]
            for ne in range(NE):
                lhsT = wqk_sb[ne][:, ff * 128:(ff + 1) * 128]
                for tt in range(NT_CHUNK):
                    nc.tensor.matmul(
                        ps[tt], lhsT,
                        xt_sb[ne][:, tt * 512:(tt + 1) * 512],
                        start=(ne == 0), stop=(ne == NE - 1),
                    )
            for tt in range(NT_CHUNK):
                nc.scalar.copy(
                    out=qk_sb[ff][:, tt * 512:(tt + 1) * 512], in_=ps[tt]
                )

        # aug assembly: heads 0,2 at partitions 0..63 (gpsimd, SBUF->SBUF);
        # heads 1,3 at 64..127 (DMA shift to base 0).
        for h in range(HPG):
            qt, kt = 2 * (h // 2), 2 * (h // 2) + 1
            if h % 2 == 0:
                nc.gpsimd.tensor_copy(out=q_aug[h][0:D, :], in_=qk_sb[qt][0:D, :])
                nc.gpsimd.tensor_copy(out=k_aug[h][0:D, :], in_=qk_sb[kt][0:D, :])
            else:
                nc.sync.dma_start(out=q_aug[h][0:D, :], in_=qk_sb[qt][D:2 * D, :])
                nc.sync.dma_start(out=k_aug[h][0:D, :], in_=qk_sb[kt][D:2 * D, :])

        # v token-major; one strided DVE copy per j-tile into vaug
        for tj in range(NT_TILE):
            psv = pvp.tile([128, DV], F32, tag="pv", name="pv")
            for ne in range(NE):
                nc.tensor.matmul(
                    psv,
                    xt_sb[ne][:, tj * 128:(tj + 1) * 128],
                    wv_sb[ne],
                    start=(ne == 0), stop=(ne == NE - 1),
                )
            ps_r = psv.rearrange("p (h c) -> p h c", c=D)
            dst = vaug.rearrange("p (h j c) -> p h j c", h=HPG, j=NT_TILE)
            nc.vector.tensor_copy(
                out=dst[:, :, tj:tj + 1, 0:D],
                in_=ps_r[:, :, None, :],
            )

        if dbg is not None and "qk0" in dbg:
            nc.sync.dma_start(out=dbg["qk0"][:, :], in_=qk_sb[0])

    # --- Phase 2+3: attention ----------------------------------------------
    with (
        tc.tile_pool(name="att", bufs=1) as att,
        tc.tile_pool(name="ps2", bufs=2, space="PSUM") as ps2,   # scores/pr/y
        tc.tile_pool(name="pod", bufs=2, space="PSUM") as pod,   # po (1/head)
    ):
        oall = [att.tile([128, T], F16, tag=f"oall{i}", name=f"oall{i}")
                for i in range(2)]
        wout_sb = [att.tile([128, E], F16, tag=f"wout{i}", name=f"wout{i}")
                   for i in range(2)]
        for i in range(2):
            nc.sync.dma_start(out=wout_sb[i], in_=woutT_d[i * 128:(i + 1) * 128, :])

        # ---- packed max pass: one head-pair per PE pass (row groups) ----
        def emit_maxpass(pxp, pair, ic):
            qt, kt = 2 * pair, 2 * pair + 1
            if True:
                mcols = [att.tile([128, NT_CHUNK], F32, tag=f"mcol{s}",
                                  name="mcol", bufs=2) for s in range(2)]
                for c in range(NT_CHUNK):
                    it = ic * NT_CHUNK + c
                    quads = [att.tile([128, NT_CHUNK], F32, tag=f"quad{s}",
                                      name="quad", bufs=2) for s in range(2)]
                    for jc in range(NT_CHUNK):
                        pxs = [pxp.tile([128, 512], F32, tag="px", name="px")
                               for _ in range(2)]
                        for s in range(2):
                            rows = slice(s * D, (s + 1) * D)
                            nc.tensor.matmul(
                                pxs[s],
                                qk_sb[qt][rows, it * 128:(it + 1) * 128],
                                qk_sb[kt][rows, jc * 512:(jc + 1) * 512],
                                start=True, stop=True,
                            )
                        for s in range(2):
                            nc.vector.reduce_max(
                                out=quads[s][:, jc:jc + 1], in_=pxs[s],
                                axis=mybir.AxisListType.X,
                            )
                    for s in range(2):
                        nc.vector.tensor_reduce(
                            out=mcols[s][:, c:c + 1], in_=quads[s],
                            axis=mybir.AxisListType.X, op=mybir.AluOpType.max,
                            negate=True,
                        )
                for s in range(2):
                    h = 2 * pair + s
                    mx4 = pxp.tile([128, 512], F32, tag="px", name="mx4")
                    nc.tensor.transpose(mx4[0:NT_CHUNK, 0:128], mcols[s], identity)
                    mstage = att.tile([NT_CHUNK, 128], F16, tag="mstage",
                                      name="mstage", bufs=2)
                    nc.vector.tensor_copy(out=mstage, in_=mx4[0:NT_CHUNK, 0:128])
                    nc.sync.dma_start(
                        out=q_aug[h][D:D + 1, ic * 512:(ic + 1) * 512].rearrange(
                            "p (c f) -> p c f", c=NT_CHUNK
                        ),
                        in_=mstage,
                    )

        # ---- main pass for a head-pair, two heads interleaved ----
        def emit_main(pair, sco_alloc, ic):
            hA, hB = 2 * pair, 2 * pair + 1
            if True:
                ics = slice(ic * 512, (ic + 1) * 512)
                pos = {}
                for h in (hA, hB):
                    pot = pod.tile([128, 512], F32, tag=f"po{h % 2}", name="po",
                                   bufs=1)
                    pos[h] = pot[0:D + 1, :]
                for jg in range(NT_TILE // 2):
                    for h in (hA, hB):
                        sco = sco_alloc()
                        pT = att.tile([128, 1024], F16, tag="pT",
                                      name="pT", bufs=4)
                        for half in range(2):
                            jt = jg * 2 + half
                            nc.tensor.matmul(
                                sco[:, half * 512:(half + 1) * 512],
                                k_aug[h][:, jt * 128:(jt + 1) * 128],
                                q_aug[h][:, ics],
                                start=True, stop=True,
                            )
                        nc.scalar.activation(
                            out=pT, in_=sco,
                            func=mybir.ActivationFunctionType.Exp,
                        )
                        if dbg is not None and h == 0 and jg == 0 and "sp0" in dbg:
                            nc.sync.dma_start(
                                out=dbg["sp0"][:, ic * 1024:(ic + 1) * 1024],
                                in_=pT,
                            )
                        for half in range(2):
                            jt = jg * 2 + half
                            nc.tensor.matmul(
                                pos[h],
                                vaug_sl(h, jt),
                                pT[:, half * 512:(half + 1) * 512],
                                start=(jt == 0), stop=(jt == NT_TILE - 1),
                            )
                for h in (hA, hB):
                    po = pos[h]
                    odd = h % 2 == 1
                    # normalize: r = exp(-ln(den)) on ACT, PE rank-1 broadcast
                    lnd = att.tile([1, 512], F32, tag="lnd", name="lnd", bufs=2)
                    nc.scalar.activation(
                        out=lnd, in_=po[D:D + 1, :],
                        func=mybir.ActivationFunctionType.Ln,
                    )
                    r16 = att.tile([1, 512], F16, tag="r16", name="r16", bufs=2)
                    nc.scalar.activation(
                        out=r16, in_=lnd,
                        func=mybir.ActivationFunctionType.Exp, scale=-1.0,
                    )
                    prt = sco_alloc()
                    pr = prt[:, 0:512]
                    nc.tensor.matmul(
                        pr[0:D, :], ones_t, r16, start=True, stop=True,
                    )
                    ot = att.tile([D, 512], F16, tag="ot", name="ot", bufs=2)
                    nc.vector.tensor_copy(out=ot, in_=po[0:D, :])
                    if not odd:
                        nc.vector.tensor_mul(
                            oall[h // 2][0:D, ics], ot, pr[0:D, :]
                        )
                    else:
                        ostage = att.tile([D, 512], F16, tag="ostage",
                                          name="ostage", bufs=2)
                        nc.vector.tensor_mul(ostage, ot, pr[0:D, :])
                        nc.sync.dma_start(
                            out=oall[h // 2][D:2 * D, ics], in_=ostage
                        )
                    if dbg is not None and h == 0 and "den0" in dbg:
                        dstage = att.tile([D + 1, 512], F32, tag="dstage",
                                          name="dstage", bufs=2)
                        nc.vector.tensor_copy(
                            out=dstage[D:D + 1, :], in_=po[D:D + 1, :]
                        )
                        nc.sync.dma_start(
                            out=dbg["den0"][ic:ic + 1, :],
                            in_=dstage[D:D + 1, :],
                        )

                # out-projection streams after the last pair finishes ic
                if pair == 1:
                    for c in range(NT_CHUNK):
                        it = ic * NT_CHUNK + c
                        yp = sco_alloc()
                        for es in range(2):
                            lhsT = oall[es][:, it * 128:(it + 1) * 128]
                            for oc in range(2):
                                nc.tensor.matmul(
                                    yp[:, oc * 512:(oc + 1) * 512], lhsT,
                                    wout_sb[es][:, oc * 512:(oc + 1) * 512],
                                    start=(es == 0), stop=(es == 1),
                                )
                        yt = att.tile([128, E], F32, tag="yt",
                                      name="yt", bufs=3)
                        nc.vector.tensor_copy(out=yt[:, 0:512], in_=yp[:, 0:512])
                        nc.scalar.copy(out=yt[:, 512:1024], in_=yp[:, 512:1024])
                        nc.sync.dma_start(
                            out=y_d[it * 128:(it + 1) * 128, :], in_=yt
                        )

        def ps2_alloc():
            return ps2.tile([128, 1024], F32, tag="s", name="s")

        with tc.tile_pool(name="pxp", bufs=2, space="PSUM") as pxp:
            for ic in range(NT_CHUNK):
                emit_maxpass(pxp, 0, ic)
            for ic in range(NT_CHUNK):
                emit_main(0, ps2_alloc, ic)
                emit_maxpass(pxp, 1, ic)
        # maxpass banks are free now: 3-deep score ring for the second half
        with tc.tile_pool(name="px2", bufs=1, space="PSUM") as px2:
            state = {"n": 0}

            def rr_alloc():
                state["n"] += 1
                if state["n"] % 3 == 0:
                    return px2.tile([128, 1024], F32, tag="s2", name="s2")
                return ps2.tile([128, 1024], F32, tag="s", name="s")

            for ic in range(NT_CHUNK):
                emit_main(1, rr_alloc, ic)

        if dbg is not None and "qaug0" in dbg:
            nc.sync.dma_start(out=dbg["qaug0"][:, :], in_=q_aug[0])
            nc.sync.dma_start(out=dbg["kaug0"][:, :], in_=k_aug[0])
        if dbg is not None and "oall0" in dbg:
            nc.sync.dma_start(out=dbg["oall0"][:, :], in_=oall[0])


def _build_nc(reps=1, debug=False):
    nc = bass.Bass()
    xT_d = nc.declare_dram_parameter("xT", [E, T], F16, isOutput=False)
    wqkT_d = nc.declare_dram_parameter("wqkT", [E, FQK], F16, isOutput=False)
    wvT_d = nc.declare_dram_parameter("wvT", [E, DV], F16, isOutput=False)
    woutT_d = nc.declare_dram_parameter("woutT", [DV, E], F16, isOutput=False)
    y_d = nc.declare_dram_parameter("y", [T, E], F32, isOutput=True)
    dram = (xT_d, wqkT_d, wvT_d, woutT_d, y_d)
    dbg = None
    if debug:
        shapes = {
            "qk0": [128, T],
            "v0": [128, NT_TILE * DV],
            "qaug0": [D + 1, T],
            "kaug0": [D + 1, T],
            "vaug0": [128, HPG * NT_TILE * (D + 1)],
            "oall0": [128, T],
            "den0": [NT_CHUNK, 512],
            "sp0": [128, 2 * T],
        }
        keys = debug if isinstance(debug, (list, tuple)) else list(shapes)
        dbg = {
            k: nc.declare_dram_parameter(k, shapes[k], F32, isOutput=True)
            for k in keys
        }
    with tile_mod.TileContext(nc) as tc:
        for _ in range(reps):
            with tc.tile_pool(name="persist", bufs=1) as persist:
                _emit_body(nc, tc, dram, {"persist": persist}, dbg=dbg)
    _split_multi_waits(nc)
    return nc


# ---------------------------------------------------------------------------
# Execution: cached jitted shard_map over 8 cores (axon/PJRT path)
_RUNNERS = {}


class _Runner:
    def __init__(self, reps=1, debug=False):
        import jax
        from jax.sharding import Mesh, PartitionSpec
        from jax.experimental.shard_map import shard_map
        from concourse import bass2jax

        bass2jax.install_neuronx_cc_hook()
        nc = self._nc = _build_nc(reps, debug=debug)

        partition_name = (
            nc.partition_id_tensor.name if nc.partition_id_tensor else None
        )
        in_names, out_names, out_avals, zero_outs = [], [], [], []
        for alloc in nc.m.functions[0].allocations:
            if not isinstance(alloc, mybir.MemoryLocationSet):
                continue
            name = alloc.memorylocations[0].name
            if alloc.kind == "ExternalInput":
                if name != partition_name:
                    in_names.append(name)
            elif alloc.kind == "ExternalOutput":
                shape = tuple(alloc.tensor_shape)
                dtype = mybir.dt.np(alloc.dtype)
                out_names.append(name)
                out_avals.append(jax.core.ShapedArray(shape, dtype))
                zero_outs.append(np.zeros(shape, dtype))
        self.in_names, self.out_names = in_names, out_names
        self.out_avals, self.zero_outs = out_avals, zero_outs
        n_params, n_outs = len(in_names), len(out_names)
        all_in_names = list(in_names) + list(out_names)
        if partition_name is not None:
            all_in_names.append(partition_name)
        all_in_names = tuple(all_in_names)

        def _body(*args):
            operands = list(args)
            if partition_name is not None:
                operands.append(bass2jax.partition_id_tensor())
            outs = bass2jax._bass_exec_p.bind(
                *operands,
                out_avals=tuple(out_avals),
                in_names=all_in_names,
                out_names=tuple(out_names),
                lowering_input_output_aliases=(),
                sim_require_finite=True,
                sim_require_nnan=True,
                nc=nc,
            )
            return tuple(outs)

        devices = jax.devices()[:N_CORES]
        assert len(devices) == N_CORES
        self.mesh = Mesh(np.asarray(devices), ("core",))
        in_specs = (PartitionSpec("core"),) * (n_params + n_outs)
        out_specs = (PartitionSpec("core"),) * n_outs
        self.donate = tuple(range(n_params, n_params + n_outs))
        self.sharded = jax.jit(
            shard_map(
                _body,
                mesh=self.mesh,
                in_specs=in_specs,
                out_specs=out_specs,
                check_rep=False,
            ),
            donate_argnums=self.donate,
            keep_unused=True,
        )

    def stage_inputs(self, per_core_in):
        import jax
        from jax.sharding import NamedSharding, PartitionSpec

        sh = NamedSharding(self.mesh, PartitionSpec("core"))
        staged = []
        for name in self.in_names:
            cat = np.concatenate(
                [np.asarray(per_core_in[c][name]) for c in range(N_CORES)], axis=0
            )
            staged.append(jax.device_put(cat, sh))
        return staged

    def fresh_outs(self):
        import jax
        from jax.sharding import NamedSharding, PartitionSpec

        sh = NamedSharding(self.mesh, PartitionSpec("core"))
        return [
            jax.device_put(
                np.zeros((N_CORES * z.shape[0], *z.shape[1:]), z.dtype), sh
            )
            for z in self.zero_outs
        ]

    def run(self, staged_in, out_bufs):
        import jax

        outs = self.sharded(*staged_in, *out_bufs)
        jax.block_until_ready(outs)
        return outs

    def results(self, outs):
        res = []
        for c in range(N_CORES):
            d = {}
            for i, name in enumerate(self.out_names):
                full = np.asarray(outs[i])
                d[name] = full.reshape(N_CORES, *self.out_avals[i].shape)[c]
            res.append(d)
        return res


def _get_runner(reps=1, debug=False):
    key = (reps, bool(debug))
    if key not in _RUNNERS:
        _RUNNERS[key] = _Runner(reps, debug=debug)
    return _RUNNERS[key]


# ---------------------------------------------------------------------------
# Host-side sharding / gather
def _per_core_inputs(x, w_qkv, w_out):
    x = np.asarray(x, dtype=np.float32)
    w_qkv = np.asarray(w_qkv, dtype=np.float32)
    w_out = np.asarray(w_out, dtype=np.float32)
    per_core = []
    for c in range(N_CORES):
        b, g = c // GROUPS, c % GROUPS
        hs = np.arange(g * HPG, (g + 1) * HPG)
        # reference qkv feature index: f = d*48 + kk*16 + h_global
        rows_q = [np.arange(D) * (3 * H_TOTAL) + h for h in hs]
        rows_k = [r + H_TOTAL for r in rows_q]
        rows_v = [r + 2 * H_TOTAL for r in rows_q]
        # wqkT column blocks: [q0 q1 k0 k1 q2 q3 k2 k3] (64 cols each)
        blocks = []
        for pair in range(2):
            blocks.append(w_qkv[rows_q[2 * pair]])
            blocks.append(w_qkv[rows_q[2 * pair + 1]])
            blocks.append(SCALE * w_qkv[rows_k[2 * pair]])
            blocks.append(SCALE * w_qkv[rows_k[2 * pair + 1]])
        wqk = np.concatenate(blocks, axis=0)          # [512, E]
        wv = np.concatenate([w_qkv[rows_v[h]] for h in range(HPG)], axis=0)
        per_core.append(
            {
                "xT": np.ascontiguousarray(x[b].T).astype(np.float16),
                "wqkT": np.ascontiguousarray(wqk.T).astype(np.float16),
                "wvT": np.ascontiguousarray(wv.T).astype(np.float16),
                "woutT": np.ascontiguousarray(
                    w_out[:, g * DV:(g + 1) * DV].T
                ).astype(np.float16),
            }
        )
    return per_core


def kernel(x, w_qkv, w_out):
    runner = _get_runner(1)
    staged = runner.stage_inputs(_per_core_inputs(x, w_qkv, w_out))
    outs = runner.run(staged, runner.fresh_outs())
    res = runner.results(outs)
    y = np.zeros((B, T, E), dtype=np.float64)
    for c in range(N_CORES):
        y[c // GROUPS] += res[c]["y"].astype(np.float64)
    return y.astype(np.float32)
